# revision 56
# baseline (speedup 1.0000x reference)
"""Tensor-parallel LlamaAttention (S=2048, HID=4096, NH=32, NKV=8) on 8 trn2 cores.

Sharding: core c owns q heads {c, c+8, c+16, c+24} (all four share kv head c)
and kv head c.  Projections + attention are fully local; avT (bf16,
[128d, 2048s] per head group) is AllGathered, then each core computes its 512
output columns of o_proj (column-parallel wo).

Design (measured ~560us vs 692us baseline on 8 axon trn2 cores):
- startup: wq/x0/wk/wv DMAs interleaved per hidden tile so the first q
  accumulation chains start within ~10us; cos/sin and later x chunks follow.
- phase-2 chunk-columns (j, C=0/1) interleaved between phase-1 chunks
  (fills x-DMA pacing gaps, spreads AllGather triggers across phase 2).
- attention inner loop: depth-3 software pipeline scores(kt) -> exp(kt) on
  ACT -> rowsum+av matmuls (kt-3); diagonal tiles width-restricted (off:512),
  no memsets.  Rowsum uses an all-ones [128,128] stationary so the result is
  pre-broadcast across partitions; normalization = reciprocal_approx_fast +
  one DVE multiply, entirely off the PE path, run inline per chunk.
- collectives: a tiny barrier AllGather pinned to the phase-1 kT tail aligns
  cores before the serial CC pipeline; per-head-group AllGathers trigger as
  each group finishes; ag_in writes ride the gpsimd software DGE so AG-gated
  loads on sync can never head-of-line block them.
- phase 3: four 4-st quarter-groups (4 PSUM banks each, two pools so 8 banks
  stay in flight) with per-head-group passes ordered by AG arrival
  (j=0,1 -> 2 -> 3), so no o_proj accumulation chain blocks the in-order PE
  queue on a late collective; agt tiles stream in quarters on sync.

Self-contained: shapes/sharding hardcoded; host does transposes/casts.
"""

from contextlib import ExitStack

import numpy as np
import ml_dtypes

import concourse.bacc as bacc
import concourse.tile as tile
import concourse.mybir as mybir
from concourse.bass_utils import run_bass_kernel_spmd

S = 2048
HID = 4096
NH = 32
NKV = 8
HD = 128
HALF = 64
N_CORES = 8
NREP = NH // NKV  # 4 q heads per core
NHT = HID // 128  # 32 hidden tiles
NST = S // 128    # 16 seq tiles
NSC = S // 512    # 4 seq chunks
BF16 = mybir.dt.bfloat16
F32 = mybir.dt.float32

_CACHE = {}


def build_nc():
    nc = bacc.Bacc("TRN2", target_bir_lowering=False, debug=False,
                   num_devices=N_CORES)

    xT = nc.dram_tensor("xT", [HID, S], BF16, kind="ExternalInput").ap()
    wq = nc.dram_tensor("wqT", [HID, NREP * HD], BF16, kind="ExternalInput").ap()
    wk = nc.dram_tensor("wkT", [HID, HD], BF16, kind="ExternalInput").ap()
    wv = nc.dram_tensor("wvT", [HID, HD], BF16, kind="ExternalInput").ap()
    wo = nc.dram_tensor("woT", [HID, 512], BF16, kind="ExternalInput").ap()
    cosT = nc.dram_tensor("cosT", [HD, S], F32, kind="ExternalInput").ap()
    sinT = nc.dram_tensor("sinT", [HD, S], F32, kind="ExternalInput").ap()
    tri = nc.dram_tensor("triT", [128, 128], BF16, kind="ExternalInput").ap()
    ones_c = nc.dram_tensor("ones_c", [128, 1], BF16, kind="ExternalInput").ap()
    ones_r = nc.dram_tensor("ones_r", [1, 128], F32, kind="ExternalInput").ap()

    o_out = nc.dram_tensor("o_out", [S, 512], F32, kind="ExternalOutput").ap()

    # groups 0/1: one full AllGather each (trigger early in the tail block).
    # groups 2/3: split into an early half (q-chunks 0-1, norms done by the
    # (j,1) block, AllGathered during phase-1 chunk 3 while the CC core is
    # idle) and a late half (q-chunks 2-3) — phase 3's early quarters then
    # never wait on a collective, and the late pieces have ~100us of margin.
    agh_in = {(j, h): nc.dram_tensor(f"agh_in{j}_{h}", [HD, S // 2],
                                     BF16).ap()
              for j in range(NREP) for h in (0, 1)}
    agh_out = {(j, h): nc.dram_tensor(f"agh_out{j}_{h}",
                                      [N_CORES * HD, S // 2], BF16,
                                      addr_space="Shared").ap()
               for j in range(NREP) for h in (0, 1)}

    with tile.TileContext(nc) as tc:
        _body(nc, tc, xT, wq, wk, wv, wo, cosT, sinT, tri,
              o_out, agh_in, agh_out)
    nc.compile()
    return nc


def _body(nc, tc, xT, wq, wk, wv, wo, cosT, sinT, tri,
          o_out, agh_in, agh_out):
    with (
        tc.tile_pool(name="consts", bufs=1) as cpool,
        tc.tile_pool(name="psum", bufs=4, space="PSUM") as psum,
        tc.tile_pool(name="qkv", bufs=1) as qkvpool,
    ):
        tri_sb = cpool.tile([128, 128], BF16, tag="tri")
        ones_sb = cpool.tile([128, 128], BF16, tag="ones")
        nc.sync.dma_start(out=tri_sb[:], in_=tri[:])
        nc.vector.memset(ones_sb[:], 1.0)

        qT_sb = [qkvpool.tile([HD, S], BF16, tag=f"qT{j}", name=f"qT{j}")
                 for j in range(NREP)]
        kT_sb = qkvpool.tile([HD, S], BF16, tag="kT")
        v_sb = qkvpool.tile([128, S], BF16, tag="v")  # col block kt = s tile kt

        with ExitStack() as es:
            ppool = es.enter_context(tc.tile_pool(name="probs", bufs=12))
            avcpool = es.enter_context(tc.tile_pool(name="avc", bufs=10))
            spool = es.enter_context(tc.tile_pool(name="small", bufs=8))
            agq = {}
            es_p2 = es.enter_context(ExitStack())
            psav = es_p2.enter_context(
                tc.tile_pool(name="psav", bufs=2, space="PSUM"))
            psrs = es_p2.enter_context(
                tc.tile_pool(name="psrs", bufs=2, space="PSUM"))
            p2 = _Phase2(nc, tc, qT_sb, kT_sb, v_sb, tri_sb, ones_sb,
                         agh_in, agh_out, agq, None,
                         ppool, avcpool, spool, psum, psav, psrs)
            with (
                tc.tile_pool(name="rconsts", bufs=1) as rcpool,
                tc.tile_pool(name="wproj", bufs=1) as wpool,
                tc.tile_pool(name="xc", bufs=64) as xpool,
                tc.tile_pool(name="rope", bufs=2) as rpool,
            ):
                p1 = _Phase1(nc, tc, xT, wq, wk, wv, cosT, sinT,
                             qT_sb, kT_sb, v_sb,
                             rcpool, wpool, xpool, rpool, psum)
                p1.issue_dmas()
                p1.chunk(0)
                p1.chunk(1)
                for j in range(NREP):
                    p2.chunk(j, 0)
                p1.chunk(2)
                for j in range(NREP):
                    p2.chunk(j, 1)
                p1.chunk(3)
            # phase-1 pools closed; open the phase-3 pools in their space
            wopool = es.enter_context(tc.tile_pool(name="wo", bufs=1))
            agpool = es.enter_context(tc.tile_pool(name="ag", bufs=3))
            opool = es.enter_context(tc.tile_pool(name="oout", bufs=4))
            p2.agpool = agpool
            # early-half AllGathers for groups 2/3: inputs were written during
            # the (j,0)/(j,1) blocks; the gpsimd queue reaches these right
            # after, so the CC core churns through them during phase-1 chunk 3
            # (also acts as the core-alignment barrier)
            for jj in range(NREP):
                nc.gpsimd.collective_compute(
                    "AllGather", mybir.AluOpType.bypass,
                    replica_groups=[list(range(N_CORES))],
                    ins=[agh_in[(jj, 0)][:]], outs=[agh_out[(jj, 0)][:]])
            # o_proj weights load during the remaining phase-2 chunks
            wo_sb = wopool.tile([128, NHT * 512], BF16, tag="wo")
            for i in range(NHT):
                nc.sync.dma_start(out=wo_sb[:, i * 512:(i + 1) * 512],
                                  in_=wo[i * 128:(i + 1) * 128, :])
            # agt prefetch at points where the AG is already complete; the
            # early-AG'd group 2/3 quarters go LAST so group 0/1's transfers
            # (needed first in phase 3) aren't queued behind their 4MB
            prefetch = {(2, 2): [(0, 0), (0, 1)], (2, 3): [(1, 0), (1, 1)],
                        (3, 2): [(2, 0), (2, 1)], (3, 3): [(3, 0), (3, 1)]}
            for j in range(NREP):
                for C in (2, 3):
                    for (jj, qq) in prefetch.get((j, C), ()):
                        p2.issue_agt(jj, qq)
                    p2.chunk(j, C)
            p2.finish()
            es_p2.close()  # free psav/psrs banks for po2
            po2 = es.enter_context(
                tc.tile_pool(name="po2", bufs=4, space="PSUM"))
            _phase3(nc, tc, wo_sb, o_out, agq, agpool,
                    psum, po2, opool, p2.issue_agt)


class _Phase1:
    def __init__(self, nc, tc, xT, wq, wk, wv, cosT, sinT,
                 qT_sb, kT_sb, v_sb, rcpool, wpool, xpool, rpool, psum):
        self.nc = nc
        self.xT, self.wq, self.wk, self.wv = xT, wq, wk, wv
        self.cosT, self.sinT = cosT, sinT
        self.qT_sb, self.kT_sb, self.v_sb = qT_sb, kT_sb, v_sb
        self.xpool, self.rpool, self.psum = xpool, rpool, psum
        self.cos_sb = rcpool.tile([HD, S], F32, tag="cos")
        self.sin_sb = rcpool.tile([HD, S], F32, tag="sin")
        self.wq_sb = wpool.tile([128, NHT * 512], BF16, tag="wq")
        self.wk_sb = wpool.tile([128, NHT * 128], BF16, tag="wk")
        self.wv_sb = wpool.tile([128, NHT * 128], BF16, tag="wv")
        self.xcs = {}

    def _x_dma(self, cs, h):
        nc = self.nc
        t = self.xpool.tile([128, 512], BF16, tag="xc", name=f"xc{cs}_{h}")
        nc.sync.dma_start(out=t[:], in_=self.xT[h * 128:(h + 1) * 128,
                                               cs * 512:(cs + 1) * 512])
        self.xcs[(cs, h)] = t

    def _cs_dma(self, cs):
        sc = slice(cs * 512, (cs + 1) * 512)
        self.nc.sync.dma_start(out=self.cos_sb[:, sc], in_=self.cosT[:, sc])
        self.nc.sync.dma_start(out=self.sin_sb[:, sc], in_=self.sinT[:, sc])

    def issue_dmas(self):
        nc = self.nc
        for h in range(NHT):
            nc.sync.dma_start(out=self.wq_sb[:, h * 512:(h + 1) * 512],
                              in_=self.wq[h * 128:(h + 1) * 128, :])
            self._x_dma(0, h)
            nc.sync.dma_start(out=self.wk_sb[:, h * 128:(h + 1) * 128],
                              in_=self.wk[h * 128:(h + 1) * 128, :])
            nc.sync.dma_start(out=self.wv_sb[:, h * 128:(h + 1) * 128],
                              in_=self.wv[h * 128:(h + 1) * 128, :])
            if h == 12:
                self._cs_dma(0)
        for h in range(NHT):
            self._x_dma(1, h)
        self._cs_dma(1)
        for h in range(NHT):
            self._x_dma(2, h)
        self._cs_dma(2)
        self._cs_dma(3)
        for h in range(NHT):
            self._x_dma(3, h)

    def chunk(self, cs):
        nc = self.nc
        sc = slice(cs * 512, (cs + 1) * 512)
        xcs, psum, rpool = self.xcs, self.psum, self.rpool
        cos_sb, sin_sb = self.cos_sb, self.sin_sb

        def _rope(dst, pp):
            t1 = rpool.tile([HALF, 512], F32, tag="t1")
            t2 = rpool.tile([HALF, 512], F32, tag="t2")
            nc.vector.tensor_mul(t1[:], pp[0:HALF, :], cos_sb[0:HALF, sc])
            nc.vector.tensor_mul(t2[:], pp[HALF:128, :], sin_sb[0:HALF, sc])
            nc.vector.tensor_sub(dst[0:HALF, sc], t1[:], t2[:])
            t3 = rpool.tile([HALF, 512], F32, tag="t1")
            t4 = rpool.tile([HALF, 512], F32, tag="t2")
            nc.vector.tensor_mul(t3[:], pp[HALF:128, :], cos_sb[HALF:128, sc])
            nc.vector.tensor_mul(t4[:], pp[0:HALF, :], sin_sb[HALF:128, sc])
            nc.vector.tensor_add(dst[HALF:128, sc], t3[:], t4[:])

        for j in range(NREP):
            pq = psum.tile([128, 512], F32, tag="mm")
            for h in range(NHT):
                nc.tensor.matmul(
                    pq[:],
                    self.wq_sb[:, h * 512 + j * 128: h * 512 + (j + 1) * 128],
                    xcs[(cs, h)][:],
                    start=(h == 0), stop=(h == NHT - 1))
            _rope(self.qT_sb[j], pq)

        pk = psum.tile([128, 512], F32, tag="mm")
        for h in range(NHT):
            nc.tensor.matmul(pk[:], self.wk_sb[:, h * 128:(h + 1) * 128],
                             xcs[(cs, h)][:],
                             start=(h == 0), stop=(h == NHT - 1))
        _rope(self.kT_sb, pk)

        pv = psum.tile([128, 512], F32, tag="mm")
        for tl in range(4):
            for h in range(NHT):
                nc.tensor.matmul(
                    pv[:, tl * 128:(tl + 1) * 128],
                    xcs[(cs, h)][:, tl * 128:(tl + 1) * 128],
                    self.wv_sb[:, h * 128:(h + 1) * 128],
                    start=(h == 0), stop=(h == NHT - 1))
        nc.scalar.copy(self.v_sb[:, sc], pv[:])


class _Phase2:
    def __init__(self, nc, tc, qT_sb, kT_sb, v_sb, tri_sb, ones_sb,
                 agh_in, agh_out, agq, agpool,
                 ppool, avcpool, spool, psum, psav, psrs):
        self.nc = nc
        self.qT_sb, self.kT_sb, self.v_sb = qT_sb, kT_sb, v_sb
        self.tri_sb, self.ones_sb = tri_sb, ones_sb
        self.agh_in, self.agh_out = agh_in, agh_out
        self.agq, self.agpool = agq, agpool
        self.ppool, self.avcpool, self.spool = ppool, avcpool, spool
        self.psum, self.psav, self.psrs = psum, psav, psrs
        self.carry = None

    def issue_agt(self, j, qq):
        nc = self.nc
        src = self.agh_out[(j, qq // 2)]
        col0 = (qq % 2) * 512
        for r in range(N_CORES):
            t = self.agpool.tile([128, 512], BF16, tag=f"ag{j}_{r}",
                                 name=f"ag{j}_{r}_{qq}")
            nc.sync.dma_start(out=t[:],
                              in_=src[r * 128:(r + 1) * 128,
                                      col0:col0 + 512])
            self.agq[(j, r, qq)] = t

    def _make_norm(self, j, C, pav, prs):
        nc = self.nc

        def f():
            qc = slice(C * 512, (C + 1) * 512)
            # prs already has the rowsum broadcast on all 128 partitions
            bsb = self.spool.tile([128, 512], F32, tag="bsb",
                                  name=f"bsb{j}_{C}")
            nc.vector.reciprocal_approx_fast(out=bsb[:], in_=prs[:])
            avc = self.avcpool.tile([128, 512], BF16, tag="avc",
                                    name=f"avc{j}_{C}")
            nc.vector.tensor_mul(avc[:], pav[:], bsb[:])
            half = C // 2
            hc = slice((C % 2) * 512, (C % 2) * 512 + 512)
            nc.gpsimd.dma_start(out=self.agh_in[(j, half)][:, hc],
                                in_=avc[:])
            if C == NSC - 1:
                # late half; the early half's AG is issued in _body
                nc.gpsimd.collective_compute(
                    "AllGather", mybir.AluOpType.bypass,
                    replica_groups=[list(range(N_CORES))],
                    ins=[self.agh_in[(j, 1)][:]],
                    outs=[self.agh_out[(j, 1)][:]])
        return f

    def chunk(self, j, C):
        nc = self.nc
        Exp = mybir.ActivationFunctionType.Exp
        qc0 = C * 512
        nkt = 4 * C + 4
        prs = self.psrs.tile([128, 512], F32, tag="rs", name=f"prs{j}_{C}")
        pav = self.psav.tile([128, 512], F32, tag="av", name=f"pav{j}_{C}")
        pend = []

        def drain_one():
            kt2, off2, pt2 = pend.pop(0)
            nc.tensor.matmul(prs[:, off2:512], self.ones_sb[:],
                             pt2[:, off2:512],
                             start=(kt2 == 0), stop=(kt2 == nkt - 1),
                             skip_group_check=True)
            nc.tensor.matmul(pav[:, off2:512],
                             self.v_sb[:, kt2 * 128:(kt2 + 1) * 128],
                             pt2[:, off2:512],
                             start=(kt2 == 0), stop=(kt2 == nkt - 1),
                             skip_group_check=True)

        for kt in range(nkt):
            off = max(0, (kt - 4 * C) * 128)  # cols < off fully masked
            ps = self.psum.tile([128, 512], F32, tag="mm",
                                name=f"ps{j}_{C}_{kt}")
            nc.tensor.matmul(ps[:, off:512],
                             self.kT_sb[:, kt * 128:(kt + 1) * 128],
                             self.qT_sb[j][:, qc0 + off: qc0 + 512],
                             start=True, stop=True)
            pt = self.ppool.tile([128, 512], BF16, tag="pt",
                                 name=f"pt{j}_{C}_{kt}")
            nc.scalar.activation(pt[:, off:512], ps[:, off:512], Exp)
            if kt >= 4 * C:
                nc.vector.tensor_mul(pt[:, off:off + 128],
                                     pt[:, off:off + 128], self.tri_sb[:])
            pend.append((kt, off, pt))
            if len(pend) > 3:
                drain_one()
        while pend:
            drain_one()
        # norm is entirely off the PE path (DVE + gpsimd), run it inline
        self._make_norm(j, C, pav, prs)()

    def finish(self):
        pass


def _phase3(nc, tc, wo_sb, o_out, agq, agpool, psum, po2, opool,
            issue_agt):
    # remaining loads: quarters 2-3 (groups 2/3 gated by the late half-AGs,
    # which have ~100us of margin before their first consumer)
    for jj in range(NREP):
        issue_agt(jj, 2)
    for jj in range(NREP):
        issue_agt(jj, 3)

    po = {}

    def open_q(g):
        pool = psum if g % 2 == 0 else po2
        tag = "mm" if g % 2 == 0 else "po2"
        for st in range(4 * g, 4 * g + 4):
            po[st] = pool.tile([128, 512], F32, tag=tag, name=f"po{st}")

    def run(g, j):
        for st in range(4 * g, 4 * g + 4):
            qq = st // 4
            c = st % 4
            for r in range(N_CORES):
                i = j * N_CORES + r
                t = agq[(j, r, qq)]
                nc.tensor.matmul(po[st][:], t[:, c * 128:(c + 1) * 128],
                                 wo_sb[:, i * 512:(i + 1) * 512],
                                 start=(i == 0), stop=(i == NHT - 1))

    def close_q(g):
        for st in range(4 * g, 4 * g + 4):
            osb = opool.tile([128, 512], F32, tag="o", name=f"o{st}")
            nc.scalar.copy(osb[:], po[st][:])
            nc.sync.dma_start(out=o_out[st * 128:(st + 1) * 128, :],
                              in_=osb[:])

    open_q(0)
    for j in range(NREP):
        run(0, j)
    close_q(0)
    open_q(1)
    for j in range(NREP):
        run(1, j)
    close_q(1)
    open_q(2)
    run(2, 0)
    open_q(3)
    run(3, 0)
    run(2, 1)
    run(3, 1)
    run(2, 2)
    run(3, 2)
    run(2, 3)
    close_q(2)
    run(3, 3)
    close_q(3)


def prep_inputs(hidden_states, wq, wk, wv, wo, cos, sin, causal_mask=None):
    bf16 = ml_dtypes.bfloat16
    x = np.asarray(hidden_states, np.float32)[0]          # (S, HID)
    xT = np.ascontiguousarray(x.T).astype(bf16)           # (HID, S)
    wq_s = (np.asarray(wq, np.float32) / np.sqrt(HD)).astype(np.float32)
    cos2 = np.asarray(cos, np.float32)[0, 0]              # (S, 64)
    sin2 = np.asarray(sin, np.float32)[0, 0]
    cosT = np.ascontiguousarray(np.concatenate([cos2.T, cos2.T], 0))  # (128, S)
    sinT = np.ascontiguousarray(np.concatenate([sin2.T, sin2.T], 0))
    kl = np.arange(128)[:, None]
    ql = np.arange(128)[None, :]
    triT = (kl <= ql).astype(bf16)                        # allow k <= q
    ones_c = np.ones((128, 1), bf16)
    ones_r = np.ones((1, 128), np.float32)

    # wo reordered to match AllGather row order: row p = j*1024 + r*128 + d
    # corresponds to head (j*8+r), dim d  ->  wo column (j*8+r)*128 + d.
    j_ = np.arange(NREP)[:, None, None]
    r_ = np.arange(N_CORES)[None, :, None]
    d_ = np.arange(HD)[None, None, :]
    col_order = ((j_ * N_CORES + r_) * HD + d_).reshape(-1)
    woT_full = np.ascontiguousarray(
        np.asarray(wo, np.float32)[:, col_order].T).astype(bf16)  # (4096c, 4096hid)

    in_maps = []
    for c in range(N_CORES):
        heads = [jj * N_CORES + c for jj in range(NREP)]
        wq_rows = np.concatenate([wq_s[h * HD:(h + 1) * HD, :] for h in heads], 0)
        wqT_c = np.ascontiguousarray(wq_rows.T).astype(bf16)        # (HID, 512)
        wkT_c = np.ascontiguousarray(
            np.asarray(wk, np.float32)[c * HD:(c + 1) * HD, :].T).astype(bf16)
        wvT_c = np.ascontiguousarray(
            np.asarray(wv, np.float32)[c * HD:(c + 1) * HD, :].T).astype(bf16)
        woT_c = np.ascontiguousarray(woT_full[:, c * 512:(c + 1) * 512])
        in_maps.append(dict(xT=xT, wqT=wqT_c, wkT=wkT_c, wvT=wvT_c, woT=woT_c,
                            cosT=cosT, sinT=sinT, triT=triT,
                            ones_c=ones_c, ones_r=ones_r))
    return in_maps


def postprocess(results):
    out = np.empty((S, HID), np.float32)
    for c in range(N_CORES):
        out[:, c * 512:(c + 1) * 512] = results[c]["o_out"]
    return out[None]


def get_nc():
    if "nc" not in _CACHE:
        _CACHE["nc"] = build_nc()
    return _CACHE["nc"]


def kernel(hidden_states, wq, wk, wv, wo, cos, sin, causal_mask=None):
    nc = get_nc()
    in_maps = prep_inputs(hidden_states, wq, wk, wv, wo, cos, sin, causal_mask)
    res = run_bass_kernel_spmd(nc, in_maps, core_ids=list(range(N_CORES)))
    return postprocess(res.results)
